# revision 2
# baseline (speedup 1.0000x reference)
"""Self-contained 8-core Trainium2 Bass kernel for MultiHeadAttention.

Problem: B=2, S=2048, D=1024, H=16 heads (hd=64), f32, self-attention
(no mask), eval mode (dropout = identity).

Sharding: data-parallel over B (2) x tensor-parallel over heads (4 groups
of 4 heads) = 8 cores. Each core computes, for its batch b and its 4
heads: Q/K/V projections (column-sliced), attention, and a partial
output projection (row-sliced Wo). Host sums the 4 partials per batch
and adds the (bv @ Wo + bo) correction (bv never enters the kernel:
ctx rows sum probs to 1, so (ctx+bv) @ Wo = ctx @ Wo + bv @ Wo).

Algebraic simplifications used (exact):
  - bk dropped: softmax over k is invariant to the per-q constant Q.bk.
  - softmax computed without max subtraction (scores bounded ~|s|<10,
    exp is safe in f32).
  - bq folded into Q^T as a per-partition bias.
  - row normalization deferred past the P@V matmul (scale ctx instead
    of probs); row sums obtained free via an appended ones-column in V.

Layouts on chip (per core):
  - x^T [D, S] (host-transposed), Q^T/K^T [head-pair(128), S] with the
    two heads of a pair stacked on partitions -> scores^T computed as
    K @ Q^T with k-positions on the output partitions (softmax
    reductions become PE-contractions), both heads of a pair running as
    concurrent K=64 row-tiled matmuls.
  - exp on ACT over 2-bank PSUM regions, output f32r.
  - PV: ctx^T[hd+1, q] = [V_h | 1]^T_k-major @ exp^T, accumulated over
    k-tiles in PSUM; row 64 is the softmax denominator.
  - matmuls run in float32r (4x faster than f32 at N>=512).
"""

import sys

sys.path.insert(0, "/opt/trn_rl_repo")

import numpy as np

B, S, D, H, HD = 2, 2048, 1024, 16, 64
HPC = 4  # heads per core
NCORES = 8
DC = D // 128  # 8 contraction chunks
ST = S // 128  # 16 s-tiles
QCW = 512  # q chunk width
QC = S // QCW  # 4 q chunks
KT = S // 128  # 16 k tiles

_CACHE = {}


def _build(repeat=1):
    import concourse.bass as bass  # noqa: F401
    import concourse.mybir as mybir
    import concourse.tile as tile
    from concourse import bacc
    from concourse.library_config import attn as attn_lib

    F32 = mybir.dt.float32
    F32R = mybir.dt.float32r
    AF = mybir.ActivationFunctionType

    nc = bacc.Bacc("TRN2", target_bir_lowering=False, debug=False)

    xt_d = nc.dram_tensor("xt", [D, S], F32R, kind="ExternalInput")
    wq_d = nc.dram_tensor("wq", [D, HPC * HD], F32R, kind="ExternalInput")
    wk_d = nc.dram_tensor("wk", [D, HPC * HD], F32R, kind="ExternalInput")
    wv_d = nc.dram_tensor("wv", [D, HPC * HD], F32R, kind="ExternalInput")
    wo_d = nc.dram_tensor("wo", [HPC * HD, D], F32R, kind="ExternalInput")
    bq_d = nc.dram_tensor("bq2", [128, 2], F32, kind="ExternalInput")
    out_d = nc.dram_tensor("out_p", [S, D], F32, kind="ExternalOutput")

    with tile.TileContext(nc) as tc:
        nc.gpsimd.load_library(attn_lib)
        with (
            tc.tile_pool(name="wp", bufs=1) as wp,
            tc.tile_pool(name="xp", bufs=1) as xp,
            tc.tile_pool(name="qk", bufs=1) as qk,
            tc.tile_pool(name="vp", bufs=1) as vp,
            tc.tile_pool(name="ep", bufs=3) as ep,
            tc.tile_pool(name="cp", bufs=1) as cp,
            tc.tile_pool(name="mp", bufs=2) as mp,
            tc.tile_pool(name="op", bufs=2) as op,
            tc.tile_pool(name="pp", bufs=2, space="PSUM") as pp,
        ):
            # ---- loads
            xt_t = xp.tile([128, DC, S], F32R, tag="xt")
            for c in range(DC):
                nc.sync.dma_start(xt_t[:, c, :], xt_d[c * 128:(c + 1) * 128, :])
            wq_t = wp.tile([128, DC, HPC * HD], F32R, tag="wq")
            nc.sync.dma_start(wq_t[:], wq_d.rearrange("(c p) n -> p c n", p=128))
            wk_t = wp.tile([128, DC, HPC * HD], F32R, tag="wk")
            nc.sync.dma_start(wk_t[:], wk_d.rearrange("(c p) n -> p c n", p=128))
            wv_t = wp.tile([128, DC, HPC * HD], F32R, tag="wv")
            nc.sync.dma_start(wv_t[:], wv_d.rearrange("(c p) n -> p c n", p=128))
            wo_t = wp.tile([128, 2, D], F32R, tag="wo")
            nc.sync.dma_start(wo_t[:], wo_d.rearrange("(c p) n -> p c n", p=128))
            bq_t = wp.tile([128, 2], F32, tag="bq")
            nc.sync.dma_start(bq_t[:], bq_d[:])
            ones_f = wp.tile([128, 64], F32, tag="onesf")
            nc.vector.memset(ones_f[:], 1.0)

            for _rep in range(repeat):
                # ---- V projection -> v1 [s, 4*(64+1)] with ones columns
                v1_t = vp.tile([128, ST, HPC * 65], F32R, tag="v1")
                with nc.allow_low_precision(reason="f32r matmul operands"):
                    nc.vector.tensor_copy(
                        v1_t[:].rearrange("p s (h c) -> p s h c", c=65)[:, :, :, 64],
                        ones_f[:, 0:64].rearrange("p (s h) -> p s h", s=ST),
                    )
                for st in range(ST):
                    vps = pp.tile([128, HPC * HD], F32, tag="qkv")
                    for c in range(DC):
                        nc.tensor.matmul(
                            vps[:],
                            xt_t[:, c, st * 128:(st + 1) * 128],
                            wv_t[:, c, :],
                            start=(c == 0),
                            stop=(c == DC - 1),
                        )
                    with nc.allow_low_precision(reason="f32r matmul operands"):
                        nc.vector.tensor_copy(
                            v1_t[:, st, :].rearrange("p (h c) -> p h c", c=65)[:, :, 0:64],
                            vps[:].rearrange("p (h c) -> p h c", c=64),
                        )

                # ---- Q^T / K^T projections (per head pair)
                qt_tiles = [qk.tile([128, S], F32R, tag=f"qt{p}", name=f"qt{p}") for p in range(2)]
                kt_tiles = [qk.tile([128, S], F32R, tag=f"kt{p}", name=f"kt{p}") for p in range(2)]

                def qkt_proj(pair):
                    for qc in range(QC):
                        qs = slice(qc * QCW, (qc + 1) * QCW)
                        kps = pp.tile([128, QCW], F32, tag="qkv")
                        for c in range(DC):
                            nc.tensor.matmul(
                                kps[:],
                                wk_t[:, c, pair * 128:(pair + 1) * 128],
                                xt_t[:, c, qs],
                                start=(c == 0),
                                stop=(c == DC - 1),
                            )
                        with nc.allow_low_precision(reason="f32r matmul operands"):
                            nc.vector.tensor_copy(kt_tiles[pair][:, qs], kps[:])
                        qps = pp.tile([128, QCW], F32, tag="qkv")
                        for c in range(DC):
                            nc.tensor.matmul(
                                qps[:],
                                wq_t[:, c, pair * 128:(pair + 1) * 128],
                                xt_t[:, c, qs],
                                start=(c == 0),
                                stop=(c == DC - 1),
                            )
                        with nc.allow_low_precision(reason="f32r matmul operands"):
                            nc.vector.tensor_scalar_add(
                                qt_tiles[pair][:, qs], qps[:], bq_t[:, pair:pair + 1]
                            )

                ctxt_tiles = [cp.tile([128, S], F32R, tag=f"ct{p}", name=f"ct{p}") for p in range(2)]

                def attention(pair, qc):
                    qs = slice(qc * QCW, (qc + 1) * QCW)
                    ctx_ps = [pp.tile([65, QCW], F32, tag="ctx", name=f"ctx{_h}") for _h in range(2)]
                    for r in range(KT):
                        sreg = pp.tile([128, 2 * QCW], F32, tag="big")
                        expt = ep.tile([128, 2 * QCW], F32R, tag="exp")
                        for h in range(2):
                            nc.tensor.matmul(
                                sreg[:, h * QCW:(h + 1) * QCW],
                                kt_tiles[pair][64 * h:64 * (h + 1), r * 128:(r + 1) * 128],
                                qt_tiles[pair][64 * h:64 * (h + 1), qs],
                                start=True,
                                stop=True,
                                tile_position=(64 * h, 0),
                            )
                        nc.scalar.activation(expt[:], sreg[:], AF.Exp, scale=0.125)
                        for h in range(2):
                            hh = 2 * pair + h
                            nc.tensor.matmul(
                                ctx_ps[h][:],
                                v1_t[:, r, 65 * hh:65 * hh + 65],
                                expt[:, h * QCW:(h + 1) * QCW],
                                start=(r == 0),
                                stop=(r == KT - 1),
                            )
                    for h in range(2):
                        rsum = mp.tile([1, QCW], F32, tag="rsum")
                        nc.vector.reciprocal(rsum[:], ctx_ps[h][64:65, :])
                        bct = mp.tile([64, QCW], F32, tag="bc")
                        nc.gpsimd.partition_broadcast(bct[:], rsum[:])
                        with nc.allow_low_precision(reason="f32r matmul operands"):
                            nc.vector.tensor_mul(
                                ctxt_tiles[pair][64 * h:64 * (h + 1), qs],
                                ctx_ps[h][0:64, :],
                                bct[:],
                            )

                def outproj(qc):
                    for sub in range(QCW // 128):
                        q0 = qc * QCW + sub * 128
                        for d2 in range(2):
                            ops = pp.tile([128, 512], F32, tag="qkv")
                            for pair in range(2):
                                nc.tensor.matmul(
                                    ops[:],
                                    ctxt_tiles[pair][:, q0:q0 + 128],
                                    wo_t[:, pair, d2 * 512:(d2 + 1) * 512],
                                    start=(pair == 0),
                                    stop=(pair == 1),
                                )
                            osb = op.tile([128, 512], F32, tag="osb")
                            nc.vector.tensor_copy(osb[:], ops[:])
                            nc.sync.dma_start(out_d[q0:q0 + 128, d2 * 512:(d2 + 1) * 512], osb[:])

                qkt_proj(0)
                for qc in range(QC):
                    attention(0, qc)
                qkt_proj(1)
                for qc in range(QC):
                    attention(1, qc)
                    outproj(qc)

    nc.compile()
    return nc


def _get_nc(repeat=1):
    if repeat not in _CACHE:
        _CACHE[repeat] = _build(repeat)
    return _CACHE[repeat]


def _make_in_maps(query_input, Wq, bq, Wk, Wv, Wo):
    x = np.asarray(query_input, dtype=np.float32)
    in_maps = []
    for core in range(NCORES):
        b, g = divmod(core, NCORES // B)
        cs = slice(g * HPC * HD, (g + 1) * HPC * HD)
        in_maps.append({
            "xt": np.ascontiguousarray(x[b].T),
            "wq": np.ascontiguousarray(Wq[:, cs]),
            "wk": np.ascontiguousarray(Wk[:, cs]),
            "wv": np.ascontiguousarray(Wv[:, cs]),
            "wo": np.ascontiguousarray(Wo[cs, :]),
            "bq2": np.ascontiguousarray(bq[cs].reshape(2, 128).T),
        })
    return in_maps


def kernel(query_input, Wq, bq, Wk, bk, Wv, bv, Wo, bo):
    from concourse.bass_utils import run_bass_kernel_spmd

    Wq = np.asarray(Wq, np.float32)
    Wk = np.asarray(Wk, np.float32)
    Wv = np.asarray(Wv, np.float32)
    Wo = np.asarray(Wo, np.float32)
    bq = np.asarray(bq, np.float32)
    bv = np.asarray(bv, np.float32)
    bo = np.asarray(bo, np.float32)

    nc = _get_nc()
    in_maps = _make_in_maps(query_input, Wq, bq, Wk, Wv, Wo)
    res = run_bass_kernel_spmd(nc, in_maps, core_ids=list(range(NCORES)))

    gpc = NCORES // B  # groups per batch
    out = np.zeros((B, S, D), np.float32)
    for core in range(NCORES):
        b = core // gpc
        out[b] += res.results[core]["out_p"]
    # bv correction (exact) + bo, applied once on the full output
    out += (bv @ Wo + bo)[None, None, :]
    return out
